# revision 1
# baseline (speedup 1.0000x reference)
"""Radix-2 DIF variant: contraction halved by pre-combining lag-product halves.

X[k, 2t]   = sum_{m<512} (R[k,m]+R[k,m+512]) * w512^{mt}
X[k, 2t+1] = sum_{m<512} (R[k,m]-R[k,m+512]) * w^m * w512^{mt}

Rsum/Rdiff are built on VectorE from sliding-window tiles (negative free-step
reads are legal on DVE), so the matmul stationaries are plain ascending slices
and the output comes out with k ascending (no J-flip on the direct path).
The w^m twiddle and the f-axis fftshift are baked into constant DFT tables
(stationary-free moving operands, resident in SBUF).
"""

import numpy as np

import bass_rust
import concourse.bass as bass
import concourse.mybir as mybir
import concourse.tile as tile
import concourse.bass_utils as bass_utils

B, N = 16, 1024
NCORES = 8
BPC = B // NCORES
NKB = 5  # k-blocks: k in [0, 640)
DS_LEN = 2176

f32 = mybir.dt.float32
f32r = mybir.dt.float32r
ALU = mybir.AluOpType


def _split_excess_waits(nc):
    for f in nc.m.functions:
        for blk in f.blocks:
            insts = list(blk.instructions)
            new_insts = []
            changed = False
            for inst in insts:
                si = inst.sync_info
                waits = list(si.on_wait) if (si is not None and si.on_wait) else []
                keep_n = 0 if isinstance(inst, mybir.InstDrain) else 1
                if len(waits) > keep_n:
                    changed = True
                    extra = waits[: len(waits) - keep_n]
                    keep = waits[len(waits) - keep_n:]
                    for w in extra:
                        nop = mybir.InstNoOp(
                            name=nc.get_next_instruction_name(), ins=[], outs=[]
                        )
                        nop.engine = inst.engine
                        nop.sync_info = bass_rust.SyncInfo(on_wait=[w], on_update=[])
                        new_insts.append(nop)
                    inst.sync_info = bass_rust.SyncInfo(
                        on_wait=keep,
                        on_update=list(si.on_update) if si.on_update else [],
                    )
                new_insts.append(inst)
            if changed:
                blk.instructions = new_insts
    return nc


TABNAMES = ["tec", "tes", "tesn", "toc", "tos", "tosn"]


def build_nc():
    nc = bass.Bass("TRN2", target_bir_lowering=False, debug=False)

    dsr = nc.dram_tensor("dsr", [BPC, DS_LEN], f32r, kind="ExternalInput")
    dsi = nc.dram_tensor("dsi", [BPC, DS_LEN], f32r, kind="ExternalInput")
    dsni = nc.dram_tensor("dsni", [BPC, DS_LEN], f32r, kind="ExternalInput")
    scols = nc.dram_tensor("scols", [BPC, 128, 16], f32, kind="ExternalInput")
    tabs = {
        nm: nc.dram_tensor(nm, [512, 512], f32r, kind="ExternalInput")
        for nm in TABNAMES
    }
    jmat = nc.dram_tensor("jmat", [128, 128], f32r, kind="ExternalInput")
    out = nc.dram_tensor("out", [BPC, N, N], f32, kind="ExternalOutput")

    with tile.TileContext(nc) as tc:
        with (
            tc.tile_pool(name="const", bufs=1) as constp,
            tc.tile_pool(name="tp", bufs=1) as tp,
            tc.tile_pool(name="rp", bufs=1) as rp,
            tc.tile_pool(name="tmp", bufs=2) as tmpp,
            tc.tile_pool(name="u", bufs=1) as up,
            tc.tile_pool(name="chi", bufs=1) as chip,
            tc.tile_pool(name="st", bufs=2) as stp,
            tc.tile_pool(name="ms", bufs=1) as msp,
            tc.tile_pool(name="mj", bufs=2) as mjp,
            tc.tile_pool(name="sm", bufs=1) as smp,
            tc.tile_pool(name="ps", bufs=2, space="PSUM") as psp,
        ):
            tJ = constp.tile([128, 128], f32r, tag="jmat")
            nc.scalar.dma_start(tJ[:], jmat[:])
            # resident DFT tables, per 128-chunk of m
            TT = {}
            k = 0
            for q in range(4):  # q-major: chunk-0 tables land first
                for nm in TABNAMES:
                    t = constp.tile([128, 512], f32r, tag=f"{nm}{q}")
                    TT[(nm, q)] = t
            def load_tab(nm, q, eng):
                eng.dma_start(TT[(nm, q)][:], tabs[nm][q * 128:(q + 1) * 128, :])

            def emit_load(b):
                s = {"b": b, "chis": [], "R": [None] * 4}
                scol = smp.tile([128, 16], f32, tag=f"scol{b}")
                nc.sync.dma_start(scol[:], scols[b])
                s["scol"] = scol
                Tsr = tp.tile([128, 1536], f32r, tag="tsr")
                Tsi = tp.tile([128, 1536], f32r, tag="tsi")
                Tnsi = tp.tile([128, 1536], f32r, tag="tnsi")
                nc.sync.dma_start(Tsr[:], bass.AP(dsr, b * DS_LEN + 385, [[1, 128], [1, 1536]]))
                nc.scalar.dma_start(Tsi[:], bass.AP(dsi, b * DS_LEN + 385, [[1, 128], [1, 1536]]))
                nc.gpsimd.dma_start(Tnsi[:], bass.AP(dsni, b * DS_LEN + 385, [[1, 128], [1, 1536]]))
                s["T"] = (Tsr, Tsi, Tnsi)
                rowall = smp.tile([1, 640], f32, tag=f"rowall{b}")
                s["rowall"] = rowall
                return s

            def win(T, off):
                # [p, kk] -> T[p, off - kk], kk in [0, 640)
                ap = T[:]
                return bass.AP(ap.tensor, ap.offset + off, [ap.ap[0], [-1, 640]])

            def emit_rbuild(s, qs, lo=0, hi=640):
                # R^T[m, kk] = s[m] * conj(s)[(m-kk)%N]; sum/diff of m and m+512.
                # Built in k-column slices so early k-blocks unblock sooner.
                Tsr, Tsi, Tnsi = s["T"]
                scol = s["scol"]
                n = hi - lo
                for q in qs:
                    m0 = 128 * q
                    terms = []
                    for half, woff in ((0, 1024 + m0), (1, 1536 + m0)):
                        sr_c = scol[:, q + 4 * half:q + 4 * half + 1]
                        si_c = scol[:, 8 + q + 4 * half:9 + q + 4 * half]
                        def w(T):
                            ap = T[:]
                            return bass.AP(ap.tensor, ap.offset + woff - 385 - lo, [ap.ap[0], [-1, n]])
                        w_sr, w_si, w_nsi = w(Tsr), w(Tsi), w(Tnsi)
                        a = tmpp.tile([128, 640], f32, tag="ta")
                        ur = up.tile([128, 640], f32, tag=f"ur{half}")
                        # Rr = sr_m*csr + si_m*si_win
                        nc.vector.tensor_scalar_mul(a[:, 0:n], w_sr, sr_c)
                        nc.vector.scalar_tensor_tensor(
                            ur[:, 0:n], w_si, si_c, a[:, 0:n], op0=ALU.mult, op1=ALU.add
                        )
                        b2 = tmpp.tile([128, 640], f32, tag="tb")
                        ui = up.tile([128, 640], f32, tag=f"ui{half}")
                        # Ri = si_m*csr - sr_m*si_win  (= si_m*csr + sr_m*(-si_win))
                        nc.vector.tensor_scalar_mul(b2[:, 0:n], w_nsi, sr_c)
                        nc.vector.scalar_tensor_tensor(
                            ui[:, 0:n], w_sr, si_c, b2[:, 0:n], op0=ALU.mult, op1=ALU.add
                        )
                        terms.append((ur, ui))
                    (u1r, u1i), (u2r, u2i) = terms
                    if lo == 0:
                        qt = f"0_{s['b'] % 2}" if q == 0 else str(q)
                        rsr = rp.tile([128, 640], f32r, tag=f"rsr{qt}")
                        rsi = rp.tile([128, 640], f32r, tag=f"rsi{qt}")
                        rdr = rp.tile([128, 640], f32r, tag=f"rdr{qt}")
                        rdi = rp.tile([128, 640], f32r, tag=f"rdi{qt}")
                    else:
                        rsr, rsi, rdr, rdi = s["R"][q]
                    nc.vector.scalar_tensor_tensor(
                        rsr[:, lo:hi], u1r[:, 0:n], 1.0, u2r[:, 0:n], op0=ALU.mult, op1=ALU.add)
                    nc.vector.scalar_tensor_tensor(
                        rdr[:, lo:hi], u1r[:, 0:n], 1.0, u2r[:, 0:n], op0=ALU.mult, op1=ALU.subtract)
                    nc.vector.scalar_tensor_tensor(
                        rsi[:, lo:hi], u1i[:, 0:n], 1.0, u2i[:, 0:n], op0=ALU.mult, op1=ALU.add)
                    nc.vector.scalar_tensor_tensor(
                        rdi[:, lo:hi], u1i[:, 0:n], 1.0, u2i[:, 0:n], op0=ALU.mult, op1=ALU.subtract)
                    s["R"][q] = (rsr, rsi, rdr, rdi)

            def emit_kblock(b, s, kb):
                c = 128 * kb
                xre = psp.tile([128, 512], f32, tag="xre")
                xie = psp.tile([128, 512], f32, tag="xie")
                xro = psp.tile([128, 512], f32, tag="xro")
                xio = psp.tile([128, 512], f32, tag="xio")
                for q in range(4):
                    rsr, rsi, rdr, rdi = s["R"][q]
                    first = q == 0
                    last = q == 3
                    psr = rsr[:, c:c + 128]
                    psi = rsi[:, c:c + 128]
                    pdr = rdr[:, c:c + 128]
                    pdi = rdi[:, c:c + 128]
                    nc.tensor.matmul(xre[:], psr, TT[("tec", q)][:], start=first, stop=False)
                    nc.tensor.matmul(xie[:], psi, TT[("tec", q)][:], start=first, stop=False)
                    nc.tensor.matmul(xro[:], pdr, TT[("toc", q)][:], start=first, stop=False)
                    nc.tensor.matmul(xio[:], pdi, TT[("toc", q)][:], start=first, stop=False)
                    nc.tensor.matmul(xre[:], psi, TT[("tes", q)][:], start=False, stop=last)
                    nc.tensor.matmul(xie[:], psr, TT[("tesn", q)][:], start=False, stop=last)
                    nc.tensor.matmul(xro[:], pdi, TT[("tos", q)][:], start=False, stop=last)
                    nc.tensor.matmul(xio[:], pdr, TT[("tosn", q)][:], start=False, stop=last)

                chi_t = chip.tile([128, N], f32, tag=f"chi{(5 * b + kb) % 6}")
                tmax2 = smp.tile([128, 2], f32, tag=f"tmax{b}")
                for parity, (xr, xi) in ((0, (xre, xie)), (1, (xro, xio))):
                    sqa = tmpp.tile([128, 512], f32, tag="ta")
                    sqb = tmpp.tile([128, 512], f32, tag="tb")
                    nc.scalar.square(sqa[:], xr[:])
                    nc.scalar.square(sqb[:], xi[:])
                    cap = chi_t[:]
                    strided = bass.AP(cap.tensor, cap.offset + parity, [cap.ap[0], [2, 512]])
                    nc.vector.tensor_add(strided, sqa[:], sqb[:])
                    nc.vector.tensor_reduce(
                        tmax2[:, parity:parity + 1], strided,
                        axis=mybir.AxisListType.X, op=ALU.max,
                    )
                tmax1 = smp.tile([128, 1], f32, tag=f"tmax1_{b}")
                nc.vector.tensor_max(tmax1[:], tmax2[:, 0:1], tmax2[:, 1:2])
                # transpose this block's per-partition max into the row
                # accumulator now, so the final reduce is one short chain
                nc.sync.dma_start(s["rowall"][0:1, 128 * kb:128 * (kb + 1)], tmax1[:])
                s["chis"].append(chi_t)

            def emit_finalize(b, s):
                gmax = smp.tile([1, 1], f32, tag=f"gmax{b}")
                nc.vector.tensor_reduce(
                    gmax[:], s["rowall"][:], axis=mybir.AxisListType.X, op=ALU.max
                )
                bmax = smp.tile([128, 1], f32, tag=f"bmax{b}")
                nc.sync.dma_start(
                    bmax[:], bass.AP(gmax[:].tensor, gmax[:].offset, [[1, 1], [0, 128]])
                )
                binv = smp.tile([128, 1], f32, tag=f"binv{b}")
                nc.vector.reciprocal(binv[:], bmax[:])
                s["binv"] = binv

            def emit_direct(b, s, kbs):
                # k is already ascending: scale + store
                binv = s["binv"]
                for kb in kbs:
                    stg = stp.tile([128, N], f32, tag="stg")
                    nc.vector.tensor_scalar_mul(stg[:], s["chis"][kb][:], binv[:])
                    r0 = (128 * kb + 512) % N
                    eng = nc.sync if kb % 2 == 0 else nc.scalar
                    eng.dma_start(out[b, r0:r0 + 128, :], stg[:])

            def emit_mirror_flip(b, s, kbs):
                # f-reverse chi[k2] rows (k2 in [1,384] live in kb 0..3)
                s.setdefault("ms", {})
                for kb in kbs:
                    chi_t = s["chis"][kb]
                    ms = msp.tile([128, N], f32r, tag=f"ms{kb % 2}")
                    ap = chi_t[:]
                    rev = bass.AP(ap.tensor, ap.offset + 1023, [ap.ap[0], [-1, 1023]])
                    nc.vector.tensor_copy(ms[:, 0:1], chi_t[:, 0:1])
                    nc.vector.tensor_copy(ms[:, 1:1024], rev)
                    s["ms"][kb] = ms

            def emit_mirror_jcopy(b, s, kbs):
                # J-flip (k asc -> desc) + unscaled PSUM->SBUF copy; no binv
                # dependency, so this overlaps the remaining k-blocks
                s.setdefault("mj", {})
                for kb in kbs:
                    ms = s["ms"][kb]
                    mj = mjp.tile([128, N], f32, tag=f"mj{kb % 2}")
                    for h in range(2):
                        hs = 512 * h
                        jy = psp.tile([128, 512], f32, tag=("xre" if h == 0 else "xro"))
                        nc.tensor.matmul(jy[:], tJ[:], ms[:, hs:hs + 512], start=True, stop=True)
                        nc.scalar.copy(mj[:, hs:hs + 512], jy[:])
                    s["mj"][kb] = mj

            def emit_mirror_store(b, s, kbs):
                # scale in place once 1/max is known, then store:
                # source partition r holds k2 = c+127-r -> dest row 385-c+r
                binv = s["binv"]
                for kb in kbs:
                    c = 128 * kb
                    mj = s["mj"][kb]
                    nc.scalar.mul(mj[:], mj[:], binv[:])
                    eng = nc.scalar if kb % 2 == 0 else nc.sync
                    if kb == 0:
                        eng.dma_start(out[b, 385:512, :], mj[0:127, :])
                    elif kb == 3:
                        eng.dma_start(out[b, 128:129, :], mj[127:128, :])
                    else:
                        r0 = 385 - c
                        eng.dma_start(out[b, r0:r0 + 128, :], mj[:])

            # --- pipelined schedule
            s0 = emit_load(0)
            for nm in TABNAMES:
                load_tab(nm, 0, nc.sync if nm in ("tec", "tes", "tesn") else nc.scalar)
            emit_rbuild(s0, [0])
            for q in (1, 2, 3):
                for i, nm in enumerate(TABNAMES):
                    load_tab(nm, q, (nc.sync, nc.scalar, nc.gpsimd)[i % 3])
            emit_rbuild(s0, [1, 2, 3], 0, 320)
            emit_rbuild(s0, [1, 2, 3], 320, 640)
            for kb in range(4):
                emit_kblock(0, s0, kb)
            s1 = emit_load(1)
            emit_rbuild(s1, [0])
            emit_kblock(0, s0, 4)
            emit_finalize(0, s0)
            emit_rbuild(s1, [1, 2, 3], 0, 320)
            emit_rbuild(s1, [1, 2, 3], 320, 640)
            emit_mirror_flip(0, s0, [0, 1])
            emit_mirror_jcopy(0, s0, [0, 1])
            emit_kblock(1, s1, 0)
            emit_kblock(1, s1, 1)
            emit_direct(0, s0, [0, 1])
            emit_mirror_store(0, s0, [0, 1])
            emit_kblock(1, s1, 2)
            emit_mirror_flip(0, s0, [2, 3])
            emit_mirror_jcopy(0, s0, [2, 3])
            emit_direct(0, s0, [2, 3])
            emit_mirror_store(0, s0, [2, 3])
            emit_kblock(1, s1, 3)
            emit_direct(0, s0, [4])
            emit_mirror_flip(1, s1, [0, 1])
            emit_mirror_jcopy(1, s1, [0, 1])
            emit_mirror_flip(1, s1, [2, 3])
            emit_mirror_jcopy(1, s1, [2, 3])
            emit_kblock(1, s1, 4)
            emit_finalize(1, s1)
            emit_direct(1, s1, [0, 1, 2, 3, 4])
            emit_mirror_store(1, s1, [0, 1, 2, 3])

    _split_excess_waits(nc)
    return nc


_NC_CACHE = {}


def _get_nc():
    if "nc" not in _NC_CACHE:
        _NC_CACHE["nc"] = build_nc()
    return _NC_CACHE["nc"]


def _get_tables():
    if "tabs" not in _NC_CACHE:
        m = np.arange(512, dtype=np.float64)[:, None]
        tp_ = np.arange(512, dtype=np.float64)[None, :]
        t_of = (tp_ + 256) % 512
        ang_e = 2.0 * np.pi * ((m * t_of) % 512) / 512
        ang_o = ang_e + 2.0 * np.pi * m / 1024
        tabs = {
            "tec": np.cos(ang_e).astype(np.float32),
            "tes": np.sin(ang_e).astype(np.float32),
            "toc": np.cos(ang_o).astype(np.float32),
            "tos": np.sin(ang_o).astype(np.float32),
        }
        tabs["tesn"] = -tabs["tes"]
        tabs["tosn"] = -tabs["tos"]
        _NC_CACHE["tabs"] = (tabs, np.eye(128, dtype=np.float32)[::-1].copy())
    return _NC_CACHE["tabs"]


def kernel(s_real: np.ndarray, s_imag: np.ndarray) -> np.ndarray:
    s_real = np.asarray(s_real, dtype=np.float32)
    s_imag = np.asarray(s_imag, dtype=np.float32)
    tabs, jnp_ = _get_tables()
    nc = _get_nc()

    in_maps = []
    for core in range(NCORES):
        sl = slice(core * BPC, (core + 1) * BPC)
        sr = s_real[sl]
        si = s_imag[sl]
        dsr = np.tile(sr, (1, 3))[:, :DS_LEN].copy()
        dsi_ = np.tile(si, (1, 3))[:, :DS_LEN].copy()
        scols = np.concatenate(
            [
                sr.reshape(BPC, 8, 128).transpose(0, 2, 1),
                si.reshape(BPC, 8, 128).transpose(0, 2, 1),
            ],
            axis=2,
        ).astype(np.float32).copy()
        im = {"dsr": dsr, "dsi": dsi_, "dsni": -dsi_, "scols": scols, "jmat": jnp_}
        im.update(tabs)
        in_maps.append(im)

    res = bass_utils.run_bass_kernel_spmd(nc, in_maps, core_ids=list(range(NCORES)))
    return np.concatenate([r["out"] for r in res.results], axis=0)



# revision 18
# speedup vs baseline: 1.2110x; 1.2110x over previous
"""Radix-2 DIF ambiguity surface, bf16 datapath.

X[k, 2t]   = sum_{m<512} (R[k,m]+R[k,m+512]) * w512^{mt}
X[k, 2t+1] = sum_{m<512} (R[k,m]-R[k,m+512]) * w^m * w512^{mt}

All matmul operands are bf16 (1 cyc/row on the PE vs 2 for f32r), built by
bf16 DVE ops (2x/4x perf modes). Normalization uses the ambiguity-function
identity chi_max = chi(0,0) = (sum |s|^2)^2: alpha = 1/sum|s|^2 is computed
on-device from the sliding-window tiles (every partition holds a full period
of the tiled signal, so a free-axis square-accumulate gives the global sum
with no partition reduction) and folded into the |X|^2 squares as (alpha*x)^2.
The k-mirror (J-flip) is done by the store DMA with a descending DRAM row
stride; the f-mirror is a single reversed DVE copy. No post-normalization
scaling, no J matmuls, no PSUM copies.
"""

import numpy as np
import ml_dtypes

import bass_rust
import concourse.bass as bass
import concourse.mybir as mybir
import concourse.tile as tile
import concourse.bass_utils as bass_utils

B, N = 16, 1024
NCORES = 8
BPC = B // NCORES
NKB = 5  # k-blocks: k in [0, 640)
DS_LEN = 2176

f32 = mybir.dt.float32
f32r = mybir.dt.float32r
bf16 = mybir.dt.bfloat16
ALU = mybir.AluOpType
ACTF = mybir.ActivationFunctionType


def _split_excess_waits(nc):
    for f in nc.m.functions:
        for blk in f.blocks:
            insts = list(blk.instructions)
            new_insts = []
            changed = False
            for inst in insts:
                si = inst.sync_info
                waits = list(si.on_wait) if (si is not None and si.on_wait) else []
                keep_n = 0 if isinstance(inst, mybir.InstDrain) else 1
                if len(waits) > keep_n:
                    changed = True
                    extra = waits[: len(waits) - keep_n]
                    keep = waits[len(waits) - keep_n:]
                    for w in extra:
                        nop = mybir.InstNoOp(
                            name=nc.get_next_instruction_name(), ins=[], outs=[]
                        )
                        nop.engine = inst.engine
                        nop.sync_info = bass_rust.SyncInfo(on_wait=[w], on_update=[])
                        new_insts.append(nop)
                    inst.sync_info = bass_rust.SyncInfo(
                        on_wait=keep,
                        on_update=list(si.on_update) if si.on_update else [],
                    )
                new_insts.append(inst)
            if changed:
                blk.instructions = new_insts
    return nc


TABNAMES = ["tec", "tes", "tesn", "toc", "tos", "tosn"]


def build_nc():
    nc = bass.Bass("TRN2", target_bir_lowering=False, debug=False)

    dsr = nc.dram_tensor("dsr", [BPC, DS_LEN], bf16, kind="ExternalInput")
    dsi = nc.dram_tensor("dsi", [BPC, DS_LEN], bf16, kind="ExternalInput")
    scols = nc.dram_tensor("scols", [BPC, 128, 24], f32, kind="ExternalInput")
    tabs = {
        nm: nc.dram_tensor(nm, [512, 512], bf16, kind="ExternalInput")
        for nm in TABNAMES
    }
    jmat = nc.dram_tensor("jmat", [128, 128], f32r, kind="ExternalInput")
    out = nc.dram_tensor("out", [BPC, N, N], f32, kind="ExternalOutput")

    with tile.TileContext(nc) as tc:
        with (
            tc.tile_pool(name="const", bufs=1) as constp,
            tc.tile_pool(name="tp", bufs=1) as tp,
            tc.tile_pool(name="rp", bufs=1) as rp,
            tc.tile_pool(name="tmp", bufs=2) as tmpp,
            tc.tile_pool(name="u", bufs=1) as up,
            tc.tile_pool(name="chi", bufs=1) as chip,
            tc.tile_pool(name="mj", bufs=2) as mjp,
            tc.tile_pool(name="sm", bufs=1) as smp,
            tc.tile_pool(name="ps", bufs=2, space="PSUM") as psp,
        ):
            tJ = constp.tile([128, 128], f32r, tag="jmat")
            nc.scalar.dma_start(tJ[:], jmat[:])
            # resident DFT tables, per 128-chunk of m
            TT = {}
            for q in range(4):  # q-major: chunk-0 tables land first
                for nm in TABNAMES:
                    t = constp.tile([128, 512], bf16, tag=f"{nm}{q}")
                    TT[(nm, q)] = t
            def load_tab(nm, q, eng):
                eng.dma_start(TT[(nm, q)][:], tabs[nm][q * 128:(q + 1) * 128, :])

            def emit_load(b):
                s = {"b": b, "chis": [], "R": [None] * 4}
                scol = smp.tile([128, 24], f32, tag=f"scol{b}")
                nc.sync.dma_start(scol[:], scols[b])
                s["scol"] = scol
                Tsr = tp.tile([128, 1536], bf16, tag="tsr")
                Tsi = tp.tile([128, 1536], bf16, tag="tsi")
                nc.sync.dma_start(Tsr[:], bass.AP(dsr, b * DS_LEN + 385, [[1, 128], [1, 1536]]))
                nc.scalar.dma_start(Tsi[:], bass.AP(dsi, b * DS_LEN + 385, [[1, 128], [1, 1536]]))
                s["T"] = (Tsr, Tsi)
                return s

            def emit_alpha(b, s):
                # alpha = 1/sum_m |s[m]|^2: each partition of the sliding
                # window tiles holds >=1 full period of the tiled signal, so a
                # free-axis square-accumulate over any 1024 columns yields the
                # global sum in every partition. No cross-partition reduce.
                Tsr, Tsi = s["T"]
                scr = tmpp.tile([128, 1024], f32, tag=f"asc{b % 2}")
                accR = smp.tile([128, 1], f32, tag=f"accR{b}")
                accI = smp.tile([128, 1], f32, tag=f"accI{b}")
                nc.scalar.activation(scr[:], Tsr[:, 512:1536], ACTF.Square,
                                     accum_out=accR[:])
                nc.scalar.activation(scr[:], Tsi[:, 512:1536], ACTF.Square,
                                     accum_out=accI[:])
                ssum = smp.tile([128, 1], f32, tag=f"ssum{b}")
                nc.vector.tensor_add(ssum[:], accR[:], accI[:])
                alpha = smp.tile([128, 1], f32, tag=f"alpha{b}")
                nc.vector.reciprocal(alpha[:], ssum[:])
                s["alpha"] = alpha

            def emit_rbuild(s, qs, lo=0, hi=640):
                # R^T[m, kk] = s[m] * conj(s)[(m-kk)%N]; sum/diff of m and m+512.
                # Built in k-column slices so early k-blocks unblock sooner.
                Tsr, Tsi = s["T"]
                scol = s["scol"]
                n = hi - lo
                for q in qs:
                    m0 = 128 * q
                    terms = []
                    for half, woff in ((0, 1024 + m0), (1, 1536 + m0)):
                        sr_c = scol[:, q + 4 * half:q + 4 * half + 1]
                        si_c = scol[:, 8 + q + 4 * half:9 + q + 4 * half]
                        nsr_c = scol[:, 16 + q + 4 * half:17 + q + 4 * half]
                        def w(T):
                            ap = T[:]
                            return bass.AP(ap.tensor, ap.offset + woff - 385 - lo, [ap.ap[0], [-1, n]])
                        w_sr, w_si = w(Tsr), w(Tsi)
                        a = tmpp.tile([128, 640], bf16, tag="ta")
                        ur = up.tile([128, 640], bf16, tag=f"ur{half}")
                        # Rr = sr_m*sr_win + si_m*si_win
                        nc.vector.tensor_scalar_mul(a[:, 0:n], w_sr, sr_c)
                        nc.vector.scalar_tensor_tensor(
                            ur[:, 0:n], w_si, si_c, a[:, 0:n], op0=ALU.mult, op1=ALU.add
                        )
                        b2 = tmpp.tile([128, 640], bf16, tag="tb")
                        ui = up.tile([128, 640], bf16, tag=f"ui{half}")
                        # Ri = si_m*sr_win - sr_m*si_win
                        nc.vector.tensor_scalar_mul(b2[:, 0:n], w_si, nsr_c)
                        nc.vector.scalar_tensor_tensor(
                            ui[:, 0:n], w_sr, si_c, b2[:, 0:n], op0=ALU.mult, op1=ALU.add
                        )
                        terms.append((ur, ui))
                    (u1r, u1i), (u2r, u2i) = terms
                    if lo == 0:
                        qt = f"0_{s['b'] % 2}" if q == 0 else str(q)
                        rsr = rp.tile([128, 640], bf16, tag=f"rsr{qt}")
                        rsi = rp.tile([128, 640], bf16, tag=f"rsi{qt}")
                        rdr = rp.tile([128, 640], bf16, tag=f"rdr{qt}")
                        rdi = rp.tile([128, 640], bf16, tag=f"rdi{qt}")
                    else:
                        rsr, rsi, rdr, rdi = s["R"][q]
                    nc.vector.scalar_tensor_tensor(
                        rsr[:, lo:hi], u1r[:, 0:n], 1.0, u2r[:, 0:n], op0=ALU.mult, op1=ALU.add)
                    nc.vector.scalar_tensor_tensor(
                        rdr[:, lo:hi], u1r[:, 0:n], 1.0, u2r[:, 0:n], op0=ALU.mult, op1=ALU.subtract)
                    nc.vector.scalar_tensor_tensor(
                        rsi[:, lo:hi], u1i[:, 0:n], 1.0, u2i[:, 0:n], op0=ALU.mult, op1=ALU.add)
                    nc.vector.scalar_tensor_tensor(
                        rdi[:, lo:hi], u1i[:, 0:n], 1.0, u2i[:, 0:n], op0=ALU.mult, op1=ALU.subtract)
                    s["R"][q] = (rsr, rsi, rdr, rdi)

            def emit_kblock(b, s, kb):
                c = 128 * kb
                xe = psp.tile([128, 1024], f32, tag="xe")
                xo = psp.tile([128, 1024], f32, tag="xo")
                xre, xie = xe[:, 0:512], xe[:, 512:1024]
                xro, xio = xo[:, 0:512], xo[:, 512:1024]
                for q in range(4):
                    rsr, rsi, rdr, rdi = s["R"][q]
                    first = q == 0
                    last = q == 3
                    psr = rsr[:, c:c + 128]
                    psi = rsi[:, c:c + 128]
                    pdr = rdr[:, c:c + 128]
                    pdi = rdi[:, c:c + 128]
                    # xe's accumulation groups close first so its square can
                    # start while xo's last matmuls still stream.
                    nc.tensor.matmul(xre, psr, TT[("tec", q)][:], start=first, stop=False)
                    nc.tensor.matmul(xie, psi, TT[("tec", q)][:], start=first, stop=False)
                    nc.tensor.matmul(xre, psi, TT[("tes", q)][:], start=False, stop=last)
                    nc.tensor.matmul(xie, psr, TT[("tesn", q)][:], start=False, stop=last)
                    nc.tensor.matmul(xro, pdr, TT[("toc", q)][:], start=first, stop=False)
                    nc.tensor.matmul(xio, pdi, TT[("toc", q)][:], start=first, stop=False)
                    nc.tensor.matmul(xro, pdi, TT[("tos", q)][:], start=False, stop=last)
                    nc.tensor.matmul(xio, pdr, TT[("tosn", q)][:], start=False, stop=last)

                chi_t = chip.tile([128, N], f32r, tag=f"chi{(5 * b + kb) % 6}")
                alpha = s["alpha"]
                for parity, x2 in ((0, xe), (1, xo)):
                    sq = tmpp.tile([128, 1024], f32, tag=f"sq{parity}")
                    # chi = (alpha*xr)^2 + (alpha*xi)^2 -- normalization folded
                    # into the activation scale.
                    nc.scalar.activation(sq[:], x2[:], ACTF.Square, scale=alpha[:])
                    cap = chi_t[:]
                    strided = bass.AP(cap.tensor, cap.offset + parity, [cap.ap[0], [2, 512]])
                    nc.vector.tensor_add(strided, sq[:, 0:512], sq[:, 512:1024])
                s["chis"].append(chi_t)

            def emit_direct(b, s, kbs):
                # k ascending: store straight from chi
                for kb in kbs:
                    r0 = (128 * kb + 512) % N
                    eng = nc.sync if kb % 2 == 0 else nc.scalar
                    eng.dma_start(out[b, r0:r0 + 128, :], s["chis"][kb][:].bitcast(f32))

            def emit_mirror_jcopy(b, s, kbs):
                # k-flip: J matmul on chi (f32r bitcast) reverses partitions;
                # the f-reversal rides the PSUM->SBUF copies.
                s.setdefault("mj", {})
                for kb in kbs:
                    chi_t = s["chis"][kb]
                    cap = chi_t[:]
                    jy = psp.tile([128, 1024], f32, tag="xe")
                    nc.tensor.matmul(jy[:, 0:512], tJ[:],
                                     chi_t[:, 0:512], start=True, stop=True)
                    nc.tensor.matmul(jy[:, 512:1024], tJ[:],
                                     chi_t[:, 512:1024], start=True, stop=True)
                    mj = mjp.tile([128, N], f32, tag=f"mj{kb % 2}")
                    jap = jy[:]
                    rev_hi = bass.AP(jap.tensor, jap.offset + 1023, [jap.ap[0], [-1, 511]])
                    rev_lo = bass.AP(jap.tensor, jap.offset + 511, [jap.ap[0], [-1, 511]])
                    nc.vector.tensor_copy(mj[:, 0:1], jy[:, 0:1])
                    nc.vector.tensor_copy(mj[:, 1:512], rev_hi)
                    nc.vector.tensor_copy(mj[:, 512:513], jy[:, 512:513])
                    nc.vector.tensor_copy(mj[:, 513:1024], rev_lo)
                    s["mj"][kb] = mj

            def emit_mirror_store(b, s, kbs):
                # mj partition r holds k = c+127-r -> dest row 385-c+r
                for kb in kbs:
                    c = 128 * kb
                    mj = s["mj"][kb]
                    eng = nc.scalar if kb % 2 == 0 else nc.sync
                    if kb == 0:
                        eng.dma_start(out[b, 385:512, :], mj[0:127, :])
                    elif kb == 3:
                        eng.dma_start(out[b, 128:129, :], mj[127:128, :])
                    else:
                        r0 = 385 - c
                        eng.dma_start(out[b, r0:r0 + 128, :], mj[:])

            # --- pipelined schedule
            s0 = emit_load(0)
            for nm in TABNAMES:
                load_tab(nm, 0, nc.sync if nm in ("tec", "tes", "tesn") else nc.scalar)
            emit_alpha(0, s0)
            emit_rbuild(s0, [0])
            for i_q, q in enumerate((1, 2, 3)):
                for i, nm in enumerate(TABNAMES):
                    load_tab(nm, q, (nc.sync, nc.scalar, nc.gpsimd)[i % 3])
            emit_rbuild(s0, [1, 2, 3], 0, 320)
            emit_rbuild(s0, [1, 2, 3], 320, 640)
            emit_kblock(0, s0, 0)
            emit_direct(0, s0, [0])
            emit_kblock(0, s0, 1)
            emit_direct(0, s0, [1])
            emit_mirror_jcopy(0, s0, [0])
            emit_mirror_store(0, s0, [0])
            emit_kblock(0, s0, 2)
            emit_direct(0, s0, [2])
            emit_mirror_jcopy(0, s0, [1])
            emit_mirror_store(0, s0, [1])
            s1 = emit_load(1)
            emit_alpha(1, s1)
            emit_rbuild(s1, [0])
            emit_kblock(0, s0, 3)
            emit_direct(0, s0, [3])
            emit_mirror_jcopy(0, s0, [2])
            emit_mirror_store(0, s0, [2])
            emit_rbuild(s1, [1, 2, 3], 0, 320)
            emit_kblock(0, s0, 4)
            emit_direct(0, s0, [4])
            emit_mirror_jcopy(0, s0, [3])
            emit_mirror_store(0, s0, [3])
            emit_rbuild(s1, [1, 2, 3], 320, 640)
            emit_kblock(1, s1, 0)
            emit_direct(1, s1, [0])
            for kb in range(1, 5):
                emit_kblock(1, s1, kb)
                emit_direct(1, s1, [kb])
                emit_mirror_jcopy(1, s1, [kb - 1])
                emit_mirror_store(1, s1, [kb - 1])

    _split_excess_waits(nc)
    return nc


_NC_CACHE = {}


def _get_nc():
    if "nc" not in _NC_CACHE:
        _NC_CACHE["nc"] = build_nc()
    return _NC_CACHE["nc"]


def _get_tables():
    if "tabs" not in _NC_CACHE:
        m = np.arange(512, dtype=np.float64)[:, None]
        tp_ = np.arange(512, dtype=np.float64)[None, :]
        t_of = (tp_ + 256) % 512
        ang_e = 2.0 * np.pi * ((m * t_of) % 512) / 512
        ang_o = ang_e + 2.0 * np.pi * m / 1024
        tabs = {
            "tec": np.cos(ang_e),
            "tes": np.sin(ang_e),
            "toc": np.cos(ang_o),
            "tos": np.sin(ang_o),
        }
        tabs["tesn"] = -tabs["tes"]
        tabs["tosn"] = -tabs["tos"]
        tabs = {k: v.astype(ml_dtypes.bfloat16) for k, v in tabs.items()}
        _NC_CACHE["tabs"] = (tabs, np.eye(128, dtype=np.float32)[::-1].copy())
    return _NC_CACHE["tabs"]


def make_in_maps(s_real: np.ndarray, s_imag: np.ndarray):
    s_real = np.asarray(s_real, dtype=np.float32)
    s_imag = np.asarray(s_imag, dtype=np.float32)
    tabs, jnp_ = _get_tables()
    in_maps = []
    for core in range(NCORES):
        sl = slice(core * BPC, (core + 1) * BPC)
        sr = s_real[sl].astype(ml_dtypes.bfloat16)
        si = s_imag[sl].astype(ml_dtypes.bfloat16)
        dsr = np.tile(sr, (1, 3))[:, :DS_LEN].copy()
        dsi_ = np.tile(si, (1, 3))[:, :DS_LEN].copy()
        scols = np.concatenate(
            [
                sr.reshape(BPC, 8, 128).transpose(0, 2, 1),
                si.reshape(BPC, 8, 128).transpose(0, 2, 1),
                (-sr).reshape(BPC, 8, 128).transpose(0, 2, 1),
            ],
            axis=2,
        ).astype(np.float32).copy()
        im = {"dsr": dsr, "dsi": dsi_, "scols": scols, "jmat": jnp_}
        im.update(tabs)
        in_maps.append(im)
    return in_maps


def kernel(s_real: np.ndarray, s_imag: np.ndarray) -> np.ndarray:
    nc = _get_nc()
    in_maps = make_in_maps(s_real, s_imag)
    res = bass_utils.run_bass_kernel_spmd(nc, in_maps, core_ids=list(range(NCORES)))
    return np.concatenate([r["out"] for r in res.results], axis=0)


# revision 20
# speedup vs baseline: 1.2964x; 1.0705x over previous
"""Radix-2 DIF ambiguity surface, fp8 DoubleRow DFT + bf16 lag products.

X[k, 2t]   = sum_{m<512} (R[k,m]+R[k,m+512]) * w512^{mt}
X[k, 2t+1] = sum_{m<512} (R[k,m]-R[k,m+512]) * w^m * w512^{mt}

The 512-point DFTs run as fp8e4m3 DoubleRow matmuls (contraction 256 per
instruction: out = W[:,0].T@X[:,0] + W[:,1].T@X[:,1], 0.5 cyc/row), so the
R chunk pairs (q, q+2) live in one [128, 2, 640] tile and the DFT tables in
[128, 2, 6*512] paired tiles. Lag products are built on the DVE from bf16
sliding-window tiles as pure tensor_scalar/tensor_tensor ops (the
scalar_tensor_tensor form with an fp32 scalar ran at <1x). Normalization
uses chi_max = chi(0,0) = (sum |s|^2)^2, computed on-device from the window
tiles (each partition holds a full period, so a free-axis square-accumulate
gives the global sum) and folded into the |X|^2 squares as (alpha*x)^2.
k-mirror via f32r J-matmul on chi; f-mirror rides the PSUM->SBUF copies.
Dummy matmuls during the load/rbuild phase hold the PE HAM at K=8/8.
"""

import numpy as np
import ml_dtypes

import bass_rust
import concourse.bass as bass
import concourse.mybir as mybir
import concourse.tile as tile
import concourse.bass_utils as bass_utils

B, N = 16, 1024
NCORES = 8
BPC = B // NCORES
NKB = 5  # k-blocks: k in [0, 640)
DS_LEN = 2176
W = 1544  # window block width (backward reads start 4B-aligned from base 384)

f32 = mybir.dt.float32
f32r = mybir.dt.float32r
bf16 = mybir.dt.bfloat16
fp8 = mybir.dt.float8e4
ALU = mybir.AluOpType
ACTF = mybir.ActivationFunctionType
PM = mybir.MatmulPerfMode


def _split_excess_waits(nc):
    for f in nc.m.functions:
        for blk in f.blocks:
            insts = list(blk.instructions)
            new_insts = []
            changed = False
            for inst in insts:
                si = inst.sync_info
                waits = list(si.on_wait) if (si is not None and si.on_wait) else []
                keep_n = 0 if isinstance(inst, mybir.InstDrain) else 1
                if len(waits) > keep_n:
                    changed = True
                    extra = waits[: len(waits) - keep_n]
                    keep = waits[len(waits) - keep_n:]
                    for w in extra:
                        nop = mybir.InstNoOp(
                            name=nc.get_next_instruction_name(), ins=[], outs=[]
                        )
                        nop.engine = inst.engine
                        nop.sync_info = bass_rust.SyncInfo(on_wait=[w], on_update=[])
                        new_insts.append(nop)
                    inst.sync_info = bass_rust.SyncInfo(
                        on_wait=keep,
                        on_update=list(si.on_update) if si.on_update else [],
                    )
                new_insts.append(inst)
            if changed:
                blk.instructions = new_insts
    return nc


TABNAMES = ["tec", "tes", "tesn", "toc", "tos", "tosn"]


def build_nc():
    nc = bass.Bass("TRN2", target_bir_lowering=False, debug=False)

    ds2 = nc.dram_tensor("ds2", [BPC, 2, DS_LEN], bf16, kind="ExternalInput")
    scols = nc.dram_tensor("scols", [BPC, 128, 24], f32, kind="ExternalInput")
    tabsp = nc.dram_tensor("tabsp", [2, 128, 2, 6 * 512], fp8, kind="ExternalInput")
    jmat = nc.dram_tensor("jmat", [128, 128], f32r, kind="ExternalInput")
    out = nc.dram_tensor("out", [BPC, N, N], f32, kind="ExternalOutput")

    with tile.TileContext(nc) as tc:
        with (
            tc.tile_pool(name="const", bufs=1) as constp,
            tc.tile_pool(name="tp", bufs=1) as tp,
            tc.tile_pool(name="rp", bufs=1) as rp,
            tc.tile_pool(name="tmp", bufs=2) as tmpp,
            tc.tile_pool(name="u", bufs=1) as up,
            tc.tile_pool(name="chi", bufs=1) as chip,
            tc.tile_pool(name="mj", bufs=2) as mjp,
            tc.tile_pool(name="sm", bufs=1) as smp,
            tc.tile_pool(name="ps", bufs=2, space="PSUM") as psp,
        ):
            tJ = constp.tile([128, 128], f32r, tag="jmat")
            nc.scalar.dma_start(tJ[:], jmat[:])
            # paired DFT tables: TTP[qp][:, sub, 512*i:512*(i+1)] holds table i
            # rows for m-chunk (qp + 2*sub)
            TTP = {}
            for qp in range(2):
                t = constp.tile([128, 2, 6 * 512], fp8, tag=f"ttp{qp}")
                TTP[qp] = t
                nc.gpsimd.dma_start(t[:], tabsp[qp])

            def tab(nm, qp):
                i = TABNAMES.index(nm)
                ap = TTP[qp][:]
                return bass.AP(ap.tensor, ap.offset + 512 * i,
                               [ap.ap[0], [6 * 512, 2], [1, 512]])

            warm = psp.tile([128, 1024], f32, tag="xe")

            def emit_load(b):
                s = {"b": b, "chis": [], "R": {}}
                scol = smp.tile([128, 24], f32, tag=f"scol{b}")
                nc.sync.dma_start(scol[:], scols[b])
                s["scol"] = scol
                ws = tp.tile([128, 2, W], bf16, tag="ws")
                nc.sync.dma_start(
                    ws[:],
                    bass.AP(ds2, (b * 2) * DS_LEN + 384, [[1, 128], [DS_LEN, 2], [1, W]]),
                )
                s["ws"] = ws
                return s

            def emit_warm(s, n):
                # dummy matmuls to hold the PE HAM unthrottled before the real
                # matmul stream starts; reads the window tile, writes a scratch
                # PSUM bank.
                ws = s["ws"]
                ap = ws[:]
                l = bass.AP(ap.tensor, ap.offset, [ap.ap[0], [1, 128]])
                r = bass.AP(ap.tensor, ap.offset, [ap.ap[0], [1, 512]])
                for _ in range(n):
                    nc.tensor.matmul(warm[:, 0:512], l, r, start=True, stop=True)

            def emit_keepalive(s, qp):
                rsr = s["R"][("rsr", qp)]
                ap = rsr[:]
                l = bass.AP(ap.tensor, ap.offset, [ap.ap[0], [1, 128]])
                tp_ap = TTP[0][:]
                r = bass.AP(tp_ap.tensor, tp_ap.offset, [tp_ap.ap[0], [1, 512]])
                nc.tensor.matmul(warm[:, 0:512], l, r, start=True, stop=True)

            def emit_alpha_act(b, s):
                # alpha = 1/sum_m |s[m]|^2: every partition of the sliding
                # window holds a full period, so a free-axis square-accumulate
                # over any 1024 columns yields the global sum per partition.
                ws = s["ws"]
                ap = ws[:]
                scr = tmpp.tile([128, 1024], f32, tag=f"asc{b % 2}")
                accR = smp.tile([128, 1], f32, tag=f"accR{b}")
                accI = smp.tile([128, 1], f32, tag=f"accI{b}")
                for blk, acc in ((0, accR), (1, accI)):
                    src = bass.AP(ap.tensor, ap.offset + blk * W + 512, [ap.ap[0], [1, 1024]])
                    nc.scalar.activation(scr[:], src, ACTF.Square, accum_out=acc[:])
                s["accs"] = (accR, accI)

            def emit_alpha_dve(b, s):
                accR, accI = s["accs"]
                ssum = smp.tile([128, 1], f32, tag=f"ssum{b}")
                nc.vector.tensor_add(ssum[:], accR[:], accI[:])
                alpha = smp.tile([128, 1], f32, tag=f"alpha{b}")
                nc.vector.reciprocal(alpha[:], ssum[:])
                s["alpha"] = alpha

            def emit_rbuild(s, lo, hi, keepalive=False):
                # R^T[m, kk] = s[m]*conj(s)[(m-kk)%N]; sum/diff of halves m and
                # m+512, written as fp8 into DoubleRow chunk-pair tiles.
                ws = s["ws"]
                wap = ws[:]
                scol = s["scol"]
                b = s["b"]
                n = hi - lo
                for q in range(4):
                    us = []
                    for h in (0, 1):
                        c8 = q + 4 * h
                        sr_c = scol[:, c8:c8 + 1]
                        si_c = scol[:, 8 + c8:9 + c8]
                        nsr_c = scol[:, 16 + c8:17 + c8]
                        j0 = 640 + 128 * q + 512 * h - lo
                        w_sr = bass.AP(wap.tensor, wap.offset + j0, [wap.ap[0], [-1, n]])
                        w_si = bass.AP(wap.tensor, wap.offset + W + j0, [wap.ap[0], [-1, n]])
                        ar = tmpp.tile([128, 640], bf16, tag="ta")
                        as_ = tmpp.tile([128, 640], bf16, tag="tb")
                        ur = up.tile([128, 640], bf16, tag=f"ur{h}")
                        nc.vector.tensor_scalar_mul(ar[:, 0:n], w_sr, sr_c)
                        nc.vector.tensor_scalar_mul(as_[:, 0:n], w_si, si_c)
                        nc.vector.tensor_add(ur[:, 0:n], ar[:, 0:n], as_[:, 0:n])
                        bi1 = tmpp.tile([128, 640], bf16, tag="tc")
                        bi2 = tmpp.tile([128, 640], bf16, tag="td")
                        ui = up.tile([128, 640], bf16, tag=f"ui{h}")
                        nc.vector.tensor_scalar_mul(bi1[:, 0:n], w_sr, si_c)
                        nc.vector.tensor_scalar_mul(bi2[:, 0:n], w_si, nsr_c)
                        nc.vector.tensor_add(ui[:, 0:n], bi1[:, 0:n], bi2[:, 0:n])
                        us.append((ur, ui))
                    (u1r, u1i), (u2r, u2i) = us
                    qp, sub = q % 2, q // 2
                    if lo == 0:
                        for nm in ("rsr", "rsi", "rdr", "rdi"):
                            if (nm, qp) not in s["R"]:
                                s["R"][(nm, qp)] = rp.tile(
                                    [128, 2, 640], fp8,
                                    name=f"{nm}{qp}b{b}", tag=f"{nm}{qp}_{b % 2}",
                                )
                    def rsl(nm):
                        ap = s["R"][(nm, qp)][:]
                        return bass.AP(ap.tensor, ap.offset + sub * 640 + lo, [ap.ap[0], [1, n]])
                    nc.vector.tensor_add(rsl("rsr"), u1r[:, 0:n], u2r[:, 0:n])
                    nc.vector.tensor_sub(rsl("rdr"), u1r[:, 0:n], u2r[:, 0:n])
                    nc.vector.tensor_add(rsl("rsi"), u1i[:, 0:n], u2i[:, 0:n])
                    nc.vector.tensor_sub(rsl("rdi"), u1i[:, 0:n], u2i[:, 0:n])
                    if keepalive and q % 2 == 1:
                        emit_keepalive(s, qp)

            def rweights(s, nm, qp, c):
                ap = s["R"][(nm, qp)][:]
                return bass.AP(ap.tensor, ap.offset + c, [ap.ap[0], [640, 2], [1, 128]])

            def emit_kblock(b, s, kb):
                c = 128 * kb
                xe = psp.tile([128, 1024], f32, tag="xe")
                xo = psp.tile([128, 1024], f32, tag="xo")
                xre, xie = xe[:, 0:512], xe[:, 512:1024]
                xro, xio = xo[:, 0:512], xo[:, 512:1024]
                for qp in range(2):
                    first = qp == 0
                    last = qp == 1
                    psr = rweights(s, "rsr", qp, c)
                    psi = rweights(s, "rsi", qp, c)
                    pdr = rweights(s, "rdr", qp, c)
                    pdi = rweights(s, "rdi", qp, c)
                    mm = lambda o, l, r, st, sp: nc.tensor.matmul(
                        o, l, r, start=st, stop=sp, perf_mode=PM.DoubleRow)
                    # xe's groups close first so its square starts while xo's
                    # last matmuls still stream
                    mm(xre, psr, tab("tec", qp), first, False)
                    mm(xie, psi, tab("tec", qp), first, False)
                    mm(xre, psi, tab("tes", qp), False, last)
                    mm(xie, psr, tab("tesn", qp), False, last)
                    mm(xro, pdr, tab("toc", qp), first, False)
                    mm(xio, pdi, tab("toc", qp), first, False)
                    mm(xro, pdi, tab("tos", qp), False, last)
                    mm(xio, pdr, tab("tosn", qp), False, last)

                chi_t = chip.tile([128, N], f32r, tag=f"chi{(5 * b + kb) % 6}")
                alpha = s["alpha"]
                for parity, x2 in ((0, xe), (1, xo)):
                    sq = tmpp.tile([128, 1024], f32, tag=f"sq{parity}")
                    # chi = (alpha*xr)^2 + (alpha*xi)^2: normalization folded
                    # into the activation scale
                    nc.scalar.activation(sq[:], x2[:], ACTF.Square, scale=alpha[:])
                    cap = chi_t[:]
                    strided = bass.AP(cap.tensor, cap.offset + parity, [cap.ap[0], [2, 512]])
                    eng = nc.gpsimd if parity == 0 else nc.vector
                    eng.tensor_add(strided, sq[:, 0:512], sq[:, 512:1024])
                s["chis"].append(chi_t)

            def emit_direct(b, s, kbs):
                for kb in kbs:
                    r0 = (128 * kb + 512) % N
                    eng = nc.sync if kb % 2 == 0 else nc.scalar
                    eng.dma_start(out[b, r0:r0 + 128, :], s["chis"][kb][:].bitcast(f32))

            def emit_mirror_jcopy(b, s, kbs):
                # k-flip: J matmul on chi reverses partitions; the f-reversal
                # rides the PSUM->SBUF copies (on ACT, which reads PSUM fast).
                s.setdefault("mj", {})
                for kb in kbs:
                    chi_t = s["chis"][kb]
                    jy = psp.tile([128, 1024], f32, tag="xe")
                    nc.tensor.matmul(jy[:, 0:512], tJ[:],
                                     chi_t[:, 0:512], start=True, stop=True)
                    nc.tensor.matmul(jy[:, 512:1024], tJ[:],
                                     chi_t[:, 512:1024], start=True, stop=True)
                    mj = mjp.tile([128, N], f32, tag=f"mj{kb % 2}")
                    jap = jy[:]
                    rev_hi = bass.AP(jap.tensor, jap.offset + 1023, [jap.ap[0], [-1, 511]])
                    rev_lo = bass.AP(jap.tensor, jap.offset + 511, [jap.ap[0], [-1, 511]])
                    nc.scalar.copy(mj[:, 0:1], jy[:, 0:1])
                    nc.scalar.copy(mj[:, 1:512], rev_hi)
                    nc.scalar.copy(mj[:, 512:513], jy[:, 512:513])
                    nc.scalar.copy(mj[:, 513:1024], rev_lo)
                    s["mj"][kb] = mj

            def emit_mirror_store(b, s, kbs):
                # mj partition r holds k = c+127-r -> dest row 385-c+r
                for kb in kbs:
                    c = 128 * kb
                    mj = s["mj"][kb]
                    eng = nc.scalar if kb % 2 == 0 else nc.sync
                    if kb == 0:
                        eng.dma_start(out[b, 385:512, :], mj[0:127, :])
                    elif kb == 3:
                        eng.dma_start(out[b, 128:129, :], mj[127:128, :])
                    else:
                        r0 = 385 - c
                        eng.dma_start(out[b, r0:r0 + 128, :], mj[:])

            # --- pipelined schedule
            s0 = emit_load(0)
            emit_warm(s0, 14)
            emit_alpha_act(0, s0)
            emit_rbuild(s0, 0, 256, keepalive=True)
            emit_alpha_dve(0, s0)
            emit_rbuild(s0, 256, 640, keepalive=True)
            emit_kblock(0, s0, 0)
            emit_direct(0, s0, [0])
            emit_kblock(0, s0, 1)
            emit_direct(0, s0, [1])
            emit_mirror_jcopy(0, s0, [0])
            emit_mirror_store(0, s0, [0])
            emit_kblock(0, s0, 2)
            emit_direct(0, s0, [2])
            emit_mirror_jcopy(0, s0, [1])
            emit_mirror_store(0, s0, [1])
            s1 = emit_load(1)
            emit_alpha_act(1, s1)
            emit_kblock(0, s0, 3)
            emit_direct(0, s0, [3])
            emit_mirror_jcopy(0, s0, [2])
            emit_mirror_store(0, s0, [2])
            emit_rbuild(s1, 0, 256)
            emit_alpha_dve(1, s1)
            emit_kblock(0, s0, 4)
            emit_direct(0, s0, [4])
            emit_mirror_jcopy(0, s0, [3])
            emit_mirror_store(0, s0, [3])
            emit_rbuild(s1, 256, 640)
            emit_kblock(1, s1, 0)
            emit_direct(1, s1, [0])
            for kb in range(1, 5):
                emit_kblock(1, s1, kb)
                emit_direct(1, s1, [kb])
                emit_mirror_jcopy(1, s1, [kb - 1])
                emit_mirror_store(1, s1, [kb - 1])

    _split_excess_waits(nc)
    return nc


_NC_CACHE = {}


def _get_nc():
    if "nc" not in _NC_CACHE:
        _NC_CACHE["nc"] = build_nc()
    return _NC_CACHE["nc"]


def _get_tables():
    if "tabs" not in _NC_CACHE:
        m = np.arange(512, dtype=np.float64)[:, None]
        tp_ = np.arange(512, dtype=np.float64)[None, :]
        t_of = (tp_ + 256) % 512
        ang_e = 2.0 * np.pi * ((m * t_of) % 512) / 512
        ang_o = ang_e + 2.0 * np.pi * m / 1024
        tabs = {
            "tec": np.cos(ang_e),
            "tes": np.sin(ang_e),
            "toc": np.cos(ang_o),
            "tos": np.sin(ang_o),
        }
        tabs["tesn"] = -tabs["tes"]
        tabs["tosn"] = -tabs["tos"]
        # paired fp8 layout: tabsp[qp, p, sub, 512*i+t] = tab_i[128*(qp+2*sub)+p, t]
        tabsp = np.zeros((2, 128, 2, 6 * 512), dtype=np.float64)
        for i, nm in enumerate(TABNAMES):
            tq = tabs[nm].reshape(4, 128, 512)  # [chunk, p, t]
            for qp in range(2):
                for sub in range(2):
                    tabsp[qp, :, sub, 512 * i:512 * (i + 1)] = tq[qp + 2 * sub]
        _NC_CACHE["tabs"] = (
            tabsp.astype(ml_dtypes.float8_e4m3),
            np.eye(128, dtype=np.float32)[::-1].copy(),
        )
    return _NC_CACHE["tabs"]


def make_in_maps(s_real: np.ndarray, s_imag: np.ndarray):
    s_real = np.asarray(s_real, dtype=np.float32)
    s_imag = np.asarray(s_imag, dtype=np.float32)
    tabsp, jnp_ = _get_tables()
    in_maps = []
    for core in range(NCORES):
        sl = slice(core * BPC, (core + 1) * BPC)
        sr = s_real[sl].astype(ml_dtypes.bfloat16)
        si = s_imag[sl].astype(ml_dtypes.bfloat16)
        ds2 = np.stack(
            [np.tile(sr, (1, 3))[:, :DS_LEN], np.tile(si, (1, 3))[:, :DS_LEN]],
            axis=1,
        ).copy()
        scols = np.concatenate(
            [
                sr.reshape(BPC, 8, 128).transpose(0, 2, 1),
                si.reshape(BPC, 8, 128).transpose(0, 2, 1),
                (-sr).reshape(BPC, 8, 128).transpose(0, 2, 1),
            ],
            axis=2,
        ).astype(np.float32).copy()
        im = {"ds2": ds2, "scols": scols, "tabsp": tabsp, "jmat": jnp_}
        in_maps.append(im)
    return in_maps


def kernel(s_real: np.ndarray, s_imag: np.ndarray) -> np.ndarray:
    nc = _get_nc()
    in_maps = make_in_maps(s_real, s_imag)
    res = bass_utils.run_bass_kernel_spmd(nc, in_maps, core_ids=list(range(NCORES)))
    return np.concatenate([r["out"] for r in res.results], axis=0)


# revision 30
# speedup vs baseline: 1.3718x; 1.0582x over previous
"""Radix-2 DIF ambiguity surface, fp8 DoubleRow DFT + bf16 lag products.

X[k, 2t]   = sum_{m<512} (R[k,m]+R[k,m+512]) * w512^{mt}
X[k, 2t+1] = sum_{m<512} (R[k,m]-R[k,m+512]) * w^m * w512^{mt}

The 512-point DFTs run as fp8e4m3 DoubleRow matmuls (contraction 256 per
instruction: out = W[:,0].T@X[:,0] + W[:,1].T@X[:,1], 0.5 cyc/row), so the
R chunk pairs (q, q+2) live in one [128, 2, 640] tile and the DFT tables in
[128, 2, 6*512] paired tiles. Lag products are built on the DVE from bf16
sliding-window tiles as pure tensor_scalar/tensor_tensor ops (the
scalar_tensor_tensor form with an fp32 scalar ran at <1x). Normalization
uses chi_max = chi(0,0) = (sum |s|^2)^2, computed on-device from the window
tiles (each partition holds a full period, so a free-axis square-accumulate
gives the global sum) and folded into the |X|^2 squares as (alpha*x)^2.
k-mirror via f32r J-matmul on chi; f-mirror rides the PSUM->SBUF copies.
Dummy matmuls during the load/rbuild phase hold the PE HAM at K=8/8.
"""

import numpy as np
import ml_dtypes

import bass_rust
import concourse.bass as bass
import concourse.mybir as mybir
import concourse.tile as tile
import concourse.bass_utils as bass_utils

B, N = 16, 1024
NCORES = 8
BPC = B // NCORES
NKB = 5  # k-blocks: k in [0, 640)
DS_LEN = 2176
W = 1544  # window block width (backward reads start 4B-aligned from base 384)

f32 = mybir.dt.float32
f32r = mybir.dt.float32r
bf16 = mybir.dt.bfloat16
fp8 = mybir.dt.float8e4
ALU = mybir.AluOpType
ACTF = mybir.ActivationFunctionType
PM = mybir.MatmulPerfMode


def _split_excess_waits(nc):
    for f in nc.m.functions:
        for blk in f.blocks:
            insts = list(blk.instructions)
            new_insts = []
            changed = False
            for inst in insts:
                si = inst.sync_info
                waits = list(si.on_wait) if (si is not None and si.on_wait) else []
                keep_n = 0 if isinstance(inst, mybir.InstDrain) else 1
                if len(waits) > keep_n:
                    changed = True
                    extra = waits[: len(waits) - keep_n]
                    keep = waits[len(waits) - keep_n:]
                    for w in extra:
                        nop = mybir.InstNoOp(
                            name=nc.get_next_instruction_name(), ins=[], outs=[]
                        )
                        nop.engine = inst.engine
                        nop.sync_info = bass_rust.SyncInfo(on_wait=[w], on_update=[])
                        new_insts.append(nop)
                    inst.sync_info = bass_rust.SyncInfo(
                        on_wait=keep,
                        on_update=list(si.on_update) if si.on_update else [],
                    )
                new_insts.append(inst)
            if changed:
                blk.instructions = new_insts
    return nc


TABNAMES = ["tec", "tes", "tesn", "toc", "tos", "tosn"]


def build_nc():
    nc = bass.Bass("TRN2", target_bir_lowering=False, debug=False)

    ds2 = nc.dram_tensor("ds2", [BPC, 2, DS_LEN], bf16, kind="ExternalInput")
    scols = nc.dram_tensor("scols", [BPC, 128, 24], f32, kind="ExternalInput")
    tabsp = nc.dram_tensor("tabsp", [2, 128, 2, 6 * 512], fp8, kind="ExternalInput")
    jmat = nc.dram_tensor("jmat", [128, 128], f32r, kind="ExternalInput")
    out = nc.dram_tensor("out", [BPC, N, N], f32, kind="ExternalOutput")

    with tile.TileContext(nc) as tc:
        with (
            tc.tile_pool(name="const", bufs=1) as constp,
            tc.tile_pool(name="tp", bufs=1) as tp,
            tc.tile_pool(name="rp", bufs=1) as rp,
            tc.tile_pool(name="tmp", bufs=2) as tmpp,
            tc.tile_pool(name="u", bufs=1) as up,
            tc.tile_pool(name="chi", bufs=1) as chip,
            tc.tile_pool(name="mj", bufs=2) as mjp,
            tc.tile_pool(name="sm", bufs=1) as smp,
            tc.tile_pool(name="ps", bufs=2, space="PSUM") as psp,
        ):
            tJ = constp.tile([128, 128], f32r, tag="jmat")
            nc.scalar.dma_start(tJ[:], jmat[:])
            # paired DFT tables: TTP[qp][:, sub, 512*i:512*(i+1)] holds table i
            # rows for m-chunk (qp + 2*sub)
            TTP = {}
            for qp in range(2):
                t = constp.tile([128, 2, 6 * 512], fp8, tag=f"ttp{qp}")
                TTP[qp] = t
                nc.gpsimd.dma_start(t[:], tabsp[qp])

            def tab(nm, qp):
                i = TABNAMES.index(nm)
                ap = TTP[qp][:]
                return bass.AP(ap.tensor, ap.offset + 512 * i,
                               [ap.ap[0], [6 * 512, 2], [1, 512]])

            warm = psp.tile([128, 1024], f32, tag="xe")

            def emit_load(b):
                s = {"b": b, "chis": [], "R": {}}
                scol = smp.tile([128, 24], f32, tag=f"scol{b}")
                nc.sync.dma_start(scol[:], scols[b])
                s["scol"] = scol
                ws = tp.tile([128, 2, W], bf16, tag="ws")
                nc.sync.dma_start(
                    ws[:],
                    bass.AP(ds2, (b * 2) * DS_LEN + 384, [[1, 128], [DS_LEN, 2], [1, W]]),
                )
                s["ws"] = ws
                return s

            def emit_warm(s, n):
                # dummy matmuls to hold the PE HAM unthrottled before the real
                # matmul stream starts; reads the window tile, writes a scratch
                # PSUM bank.
                ws = s["ws"]
                ap = ws[:]
                l = bass.AP(ap.tensor, ap.offset, [ap.ap[0], [1, 128]])
                r = bass.AP(ap.tensor, ap.offset, [ap.ap[0], [1, 512]])
                for _ in range(n):
                    nc.tensor.matmul(warm[:, 0:512], l, r, start=True, stop=True)

            def emit_keepalive(dep_tile, mov_tile=None):
                # dummy matmul whose stationary reads a freshly-written tile,
                # so it lands spaced through the producing stream and keeps
                # the PE HAM active
                ap = dep_tile[:]
                l = bass.AP(ap.tensor, ap.offset, [ap.ap[0], [1, 128]])
                mp = (mov_tile if mov_tile is not None else TTP[0])[:]
                r = bass.AP(mp.tensor, mp.offset, [mp.ap[0], [1, 512]])
                nc.tensor.matmul(warm[:, 0:512], l, r, start=True, stop=True)

            def emit_alpha_act(b, s):
                # alpha = 1/sum_m |s[m]|^2: every partition of the sliding
                # window holds a full period, so a free-axis square-accumulate
                # over any 1024 columns yields the global sum per partition.
                ws = s["ws"]
                ap = ws[:]
                scr = tmpp.tile([128, 1024], f32, tag=f"asc{b % 2}")
                accR = smp.tile([128, 1], f32, tag=f"accR{b}")
                accI = smp.tile([128, 1], f32, tag=f"accI{b}")
                for blk, acc in ((0, accR), (1, accI)):
                    src = bass.AP(ap.tensor, ap.offset + blk * W + 512, [ap.ap[0], [1, 1024]])
                    nc.scalar.activation(scr[:], src, ACTF.Square, accum_out=acc[:])
                s["accs"] = (accR, accI)

            def emit_alpha_dve(b, s):
                accR, accI = s["accs"]
                ssum = smp.tile([128, 1], f32, tag=f"ssum{b}")
                nc.vector.tensor_add(ssum[:], accR[:], accI[:])
                alpha = smp.tile([128, 1], f32, tag=f"alpha{b}")
                nc.vector.reciprocal(alpha[:], ssum[:])
                s["alpha"] = alpha

            def emit_rbuild(s, lo, hi, keepalive=False):
                # R^T[m, kk] = s[m]*conj(s)[(m-kk)%N]; sum/diff of halves m and
                # m+512, written as fp8 into DoubleRow chunk-pair tiles.
                ws = s["ws"]
                wap = ws[:]
                scol = s["scol"]
                b = s["b"]
                n = hi - lo
                for q in range(4):
                    us = []
                    for h in (0, 1):
                        c8 = q + 4 * h
                        sr_c = scol[:, c8:c8 + 1]
                        si_c = scol[:, 8 + c8:9 + c8]
                        nsr_c = scol[:, 16 + c8:17 + c8]
                        j0 = 640 + 128 * q + 512 * h - lo
                        w_sr = bass.AP(wap.tensor, wap.offset + j0, [wap.ap[0], [-1, n]])
                        w_si = bass.AP(wap.tensor, wap.offset + W + j0, [wap.ap[0], [-1, n]])
                        a = tmpp.tile([128, 640], bf16, tag="ta")
                        ur = up.tile([128, 640], bf16, tag=f"ur{h}")
                        # Rr = sr_m*sr_win + si_m*si_win
                        nc.vector.tensor_scalar_mul(a[:, 0:n], w_sr, sr_c)
                        nc.vector.scalar_tensor_tensor(
                            ur[:, 0:n], w_si, si_c, a[:, 0:n], op0=ALU.mult, op1=ALU.add)
                        b2 = tmpp.tile([128, 640], bf16, tag="tb")
                        ui = up.tile([128, 640], bf16, tag=f"ui{h}")
                        # Ri = si_m*sr_win - sr_m*si_win
                        nc.vector.tensor_scalar_mul(b2[:, 0:n], w_si, nsr_c)
                        nc.vector.scalar_tensor_tensor(
                            ui[:, 0:n], w_sr, si_c, b2[:, 0:n], op0=ALU.mult, op1=ALU.add)
                        if keepalive:
                            emit_keepalive(ui, mov_tile=ws)
                        us.append((ur, ui))
                    (u1r, u1i), (u2r, u2i) = us
                    qp, sub = q % 2, q // 2
                    if lo == 0:
                        for nm in ("rsr", "rsi", "rdr", "rdi"):
                            if (nm, qp) not in s["R"]:
                                s["R"][(nm, qp)] = rp.tile(
                                    [128, 2, 640], fp8,
                                    name=f"{nm}{qp}b{b}", tag=f"{nm}{qp}_{b % 2}",
                                )
                    def rsl(nm):
                        ap = s["R"][(nm, qp)][:]
                        return bass.AP(ap.tensor, ap.offset + sub * 640 + lo, [ap.ap[0], [1, n]])
                    nc.vector.tensor_add(rsl("rsr"), u1r[:, 0:n], u2r[:, 0:n])
                    nc.vector.tensor_sub(rsl("rdr"), u1r[:, 0:n], u2r[:, 0:n])
                    nc.vector.tensor_add(rsl("rsi"), u1i[:, 0:n], u2i[:, 0:n])
                    nc.vector.tensor_sub(rsl("rdi"), u1i[:, 0:n], u2i[:, 0:n])
                    if keepalive:
                        emit_keepalive(s["R"][("rdi", qp)])

            def rweights(s, nm, qp, c):
                ap = s["R"][(nm, qp)][:]
                return bass.AP(ap.tensor, ap.offset + c, [ap.ap[0], [640, 2], [1, 128]])

            def emit_kblock(b, s, kb):
                c = 128 * kb
                xe = psp.tile([128, 1024], f32, tag="xe")
                xo = psp.tile([128, 1024], f32, tag="xo")
                xre, xie = xe[:, 0:512], xe[:, 512:1024]
                xro, xio = xo[:, 0:512], xo[:, 512:1024]
                for qp in range(2):
                    first = qp == 0
                    last = qp == 1
                    psr = rweights(s, "rsr", qp, c)
                    psi = rweights(s, "rsi", qp, c)
                    pdr = rweights(s, "rdr", qp, c)
                    pdi = rweights(s, "rdi", qp, c)
                    mm = lambda o, l, r, st, sp: nc.tensor.matmul(
                        o, l, r, start=st, stop=sp, perf_mode=PM.DoubleRow)
                    # xe's groups close first so its square starts while xo's
                    # last matmuls still stream
                    mm(xre, psr, tab("tec", qp), first, False)
                    mm(xie, psi, tab("tec", qp), first, False)
                    mm(xre, psi, tab("tes", qp), False, last)
                    mm(xie, psr, tab("tesn", qp), False, last)
                    mm(xro, pdr, tab("toc", qp), first, False)
                    mm(xio, pdi, tab("toc", qp), first, False)
                    mm(xro, pdi, tab("tos", qp), False, last)
                    mm(xio, pdr, tab("tosn", qp), False, last)

                chi_t = chip.tile([128, N], f32r, tag=f"chi{(5 * b + kb) % 6}")
                alpha = s["alpha"]
                for parity, x2 in ((0, xe), (1, xo)):
                    sq = tmpp.tile([128, 1024], f32, tag=f"sq{parity}")
                    # chi = (alpha*xr)^2 + (alpha*xi)^2: normalization folded
                    # into the activation scale
                    nc.scalar.activation(sq[:], x2[:], ACTF.Square, scale=alpha[:])
                    cap = chi_t[:]
                    strided = bass.AP(cap.tensor, cap.offset + parity, [cap.ap[0], [2, 512]])
                    nc.gpsimd.tensor_add(strided, sq[:, 0:512], sq[:, 512:1024])
                s["chis"].append(chi_t)

            def emit_direct(b, s, kbs):
                for kb in kbs:
                    r0 = (128 * kb + 512) % N
                    nc.sync.dma_start(out[b, r0:r0 + 128, :], s["chis"][kb][:].bitcast(f32))

            def emit_mirror_jcopy(b, s, kbs):
                # k-flip: J matmul on chi reverses partitions; the f-reversal
                # rides the PSUM->SBUF copies (on ACT, which reads PSUM fast).
                s.setdefault("mj", {})
                for kb in kbs:
                    chi_t = s["chis"][kb]
                    jy = psp.tile([128, 1024], f32, tag="xe")
                    nc.tensor.matmul(jy[:, 0:512], tJ[:],
                                     chi_t[:, 0:512], start=True, stop=True)
                    nc.tensor.matmul(jy[:, 512:1024], tJ[:],
                                     chi_t[:, 512:1024], start=True, stop=True)
                    mj = mjp.tile([128, N], f32, tag=f"mj{kb % 2}")
                    jap = jy[:]
                    rev_hi = bass.AP(jap.tensor, jap.offset + 1023, [jap.ap[0], [-1, 511]])
                    rev_lo = bass.AP(jap.tensor, jap.offset + 511, [jap.ap[0], [-1, 511]])
                    nc.scalar.copy(mj[:, 0:1], jy[:, 0:1])
                    nc.scalar.copy(mj[:, 1:512], rev_hi)
                    nc.scalar.copy(mj[:, 512:513], jy[:, 512:513])
                    nc.scalar.copy(mj[:, 513:1024], rev_lo)
                    s["mj"][kb] = mj

            def emit_mirror_store(b, s, kbs):
                # mj partition r holds k = c+127-r -> dest row 385-c+r
                for kb in kbs:
                    c = 128 * kb
                    mj = s["mj"][kb]
                    eng = nc.sync
                    if kb == 0:
                        eng.dma_start(out[b, 385:512, :], mj[0:127, :])
                    elif kb == 3:
                        eng.dma_start(out[b, 128:129, :], mj[127:128, :])
                    else:
                        r0 = 385 - c
                        eng.dma_start(out[b, r0:r0 + 128, :], mj[:])

            # --- pipelined schedule: DVE runs the two rbuilds back-to-back;
            # the PE streams batch-0 kblocks against rbuild(s1); warm/keepalive
            # matmuls bridge the PE-idle stretches so the HAM stays at 8/8.
            s0 = emit_load(0)
            emit_warm(s0, 14)
            emit_alpha_act(0, s0)
            emit_rbuild(s0, 0, 640, keepalive=True)
            emit_alpha_dve(0, s0)
            s1 = emit_load(1)
            emit_alpha_act(1, s1)
            emit_kblock(0, s0, 0)
            emit_direct(0, s0, [0])
            emit_rbuild(s1, 0, 640)
            emit_alpha_dve(1, s1)
            emit_kblock(0, s0, 1)
            emit_direct(0, s0, [1])
            emit_mirror_jcopy(0, s0, [0])
            emit_mirror_store(0, s0, [0])
            emit_kblock(0, s0, 2)
            emit_direct(0, s0, [2])
            emit_mirror_jcopy(0, s0, [1])
            emit_mirror_store(0, s0, [1])
            emit_kblock(0, s0, 3)
            emit_direct(0, s0, [3])
            emit_mirror_jcopy(0, s0, [2])
            emit_mirror_store(0, s0, [2])
            emit_kblock(0, s0, 4)
            emit_direct(0, s0, [4])
            emit_mirror_jcopy(0, s0, [3])
            emit_mirror_store(0, s0, [3])
            # bridge the gap until rbuild(s1) completes, one keepalive per
            # R array so the deps land spread across its combine stream
            for nm in ("rsr", "rsi", "rdr", "rdi"):
                ap = s1["R"][(nm, 1)][:]
                l = bass.AP(ap.tensor, ap.offset, [ap.ap[0], [1, 128]])
                tp_ap = TTP[0][:]
                r = bass.AP(tp_ap.tensor, tp_ap.offset, [tp_ap.ap[0], [1, 512]])
                nc.tensor.matmul(warm[:, 0:512], l, r, start=True, stop=True)
            emit_kblock(1, s1, 0)
            emit_direct(1, s1, [0])
            for kb in range(1, 5):
                emit_kblock(1, s1, kb)
                emit_direct(1, s1, [kb])
                emit_mirror_jcopy(1, s1, [kb - 1])
                emit_mirror_store(1, s1, [kb - 1])

    _split_excess_waits(nc)
    return nc


_NC_CACHE = {}


def _get_nc():
    if "nc" not in _NC_CACHE:
        _NC_CACHE["nc"] = build_nc()
    return _NC_CACHE["nc"]


def _get_tables():
    if "tabs" not in _NC_CACHE:
        m = np.arange(512, dtype=np.float64)[:, None]
        tp_ = np.arange(512, dtype=np.float64)[None, :]
        t_of = (tp_ + 256) % 512
        ang_e = 2.0 * np.pi * ((m * t_of) % 512) / 512
        ang_o = ang_e + 2.0 * np.pi * m / 1024
        tabs = {
            "tec": np.cos(ang_e),
            "tes": np.sin(ang_e),
            "toc": np.cos(ang_o),
            "tos": np.sin(ang_o),
        }
        tabs["tesn"] = -tabs["tes"]
        tabs["tosn"] = -tabs["tos"]
        # paired fp8 layout: tabsp[qp, p, sub, 512*i+t] = tab_i[128*(qp+2*sub)+p, t]
        tabsp = np.zeros((2, 128, 2, 6 * 512), dtype=np.float64)
        for i, nm in enumerate(TABNAMES):
            tq = tabs[nm].reshape(4, 128, 512)  # [chunk, p, t]
            for qp in range(2):
                for sub in range(2):
                    tabsp[qp, :, sub, 512 * i:512 * (i + 1)] = tq[qp + 2 * sub]
        _NC_CACHE["tabs"] = (
            tabsp.astype(ml_dtypes.float8_e4m3),
            np.eye(128, dtype=np.float32)[::-1].copy(),
        )
    return _NC_CACHE["tabs"]


def make_in_maps(s_real: np.ndarray, s_imag: np.ndarray):
    s_real = np.asarray(s_real, dtype=np.float32)
    s_imag = np.asarray(s_imag, dtype=np.float32)
    tabsp, jnp_ = _get_tables()
    in_maps = []
    for core in range(NCORES):
        sl = slice(core * BPC, (core + 1) * BPC)
        sr = s_real[sl].astype(ml_dtypes.bfloat16)
        si = s_imag[sl].astype(ml_dtypes.bfloat16)
        ds2 = np.stack(
            [np.tile(sr, (1, 3))[:, :DS_LEN], np.tile(si, (1, 3))[:, :DS_LEN]],
            axis=1,
        ).copy()
        scols = np.concatenate(
            [
                sr.reshape(BPC, 8, 128).transpose(0, 2, 1),
                si.reshape(BPC, 8, 128).transpose(0, 2, 1),
                (-sr).reshape(BPC, 8, 128).transpose(0, 2, 1),
            ],
            axis=2,
        ).astype(np.float32).copy()
        im = {"ds2": ds2, "scols": scols, "tabsp": tabsp, "jmat": jnp_}
        in_maps.append(im)
    return in_maps


def kernel(s_real: np.ndarray, s_imag: np.ndarray) -> np.ndarray:
    nc = _get_nc()
    in_maps = make_in_maps(s_real, s_imag)
    res = bass_utils.run_bass_kernel_spmd(nc, in_maps, core_ids=list(range(NCORES)))
    return np.concatenate([r["out"] for r in res.results], axis=0)


# revision 35
# speedup vs baseline: 1.4084x; 1.0266x over previous
"""Radix-2 DIF ambiguity surface, fp8 DoubleRow DFT + bf16 lag products.

X[k, 2t]   = sum_{m<512} (R[k,m]+R[k,m+512]) * w512^{mt}
X[k, 2t+1] = sum_{m<512} (R[k,m]-R[k,m+512]) * w^m * w512^{mt}

The 512-point DFTs run as fp8e4m3 DoubleRow matmuls (contraction 256 per
instruction: out = W[:,0].T@X[:,0] + W[:,1].T@X[:,1], 0.5 cyc/row), so the
R chunk pairs (q, q+2) live in one [128, 2, 640] tile and the DFT tables in
[128, 2, 6*512] paired tiles. Lag products are built on the DVE from bf16
sliding-window tiles as pure tensor_scalar/tensor_tensor ops (the
scalar_tensor_tensor form with an fp32 scalar ran at <1x). Normalization
uses chi_max = chi(0,0) = (sum |s|^2)^2, computed on-device from the window
tiles (each partition holds a full period, so a free-axis square-accumulate
gives the global sum) and folded into the |X|^2 squares as (alpha*x)^2.
k-mirror via f32r J-matmul on chi; f-mirror rides the PSUM->SBUF copies.
Dummy matmuls during the load/rbuild phase hold the PE HAM at K=8/8.
"""

import numpy as np
import ml_dtypes

import bass_rust
import concourse.bass as bass
import concourse.mybir as mybir
import concourse.tile as tile
import concourse.bass_utils as bass_utils

B, N = 16, 1024
NCORES = 8
BPC = B // NCORES
NKB = 5  # k-blocks: k in [0, 640)
DS_LEN = 2176
W = 1544  # window block width (backward reads start 4B-aligned from base 384)

f32 = mybir.dt.float32
f32r = mybir.dt.float32r
bf16 = mybir.dt.bfloat16
fp8 = mybir.dt.float8e4
ALU = mybir.AluOpType
ACTF = mybir.ActivationFunctionType
PM = mybir.MatmulPerfMode


def _split_excess_waits(nc):
    for f in nc.m.functions:
        for blk in f.blocks:
            insts = list(blk.instructions)
            new_insts = []
            changed = False
            for inst in insts:
                si = inst.sync_info
                waits = list(si.on_wait) if (si is not None and si.on_wait) else []
                keep_n = 0 if isinstance(inst, mybir.InstDrain) else 1
                if len(waits) > keep_n:
                    changed = True
                    extra = waits[: len(waits) - keep_n]
                    keep = waits[len(waits) - keep_n:]
                    for w in extra:
                        nop = mybir.InstNoOp(
                            name=nc.get_next_instruction_name(), ins=[], outs=[]
                        )
                        nop.engine = inst.engine
                        nop.sync_info = bass_rust.SyncInfo(on_wait=[w], on_update=[])
                        new_insts.append(nop)
                    inst.sync_info = bass_rust.SyncInfo(
                        on_wait=keep,
                        on_update=list(si.on_update) if si.on_update else [],
                    )
                new_insts.append(inst)
            if changed:
                blk.instructions = new_insts
    return nc


TABNAMES = ["tec", "tes", "tesn", "toc", "tos", "tosn"]


def build_nc():
    nc = bass.Bass("TRN2", target_bir_lowering=False, debug=False)

    ds2 = nc.dram_tensor("ds2", [BPC, 2, DS_LEN], bf16, kind="ExternalInput")
    scols = nc.dram_tensor("scols", [BPC, 128, 24], f32, kind="ExternalInput")
    tabsp = nc.dram_tensor("tabsp", [2, 128, 2, 6 * 512], fp8, kind="ExternalInput")
    jmat = nc.dram_tensor("jmat", [128, 128], f32r, kind="ExternalInput")
    out = nc.dram_tensor("out", [BPC, N, N], f32, kind="ExternalOutput")

    with tile.TileContext(nc) as tc:
        with (
            tc.tile_pool(name="const", bufs=1) as constp,
            tc.tile_pool(name="tp", bufs=1) as tp,
            tc.tile_pool(name="rp", bufs=1) as rp,
            tc.tile_pool(name="tmp", bufs=2) as tmpp,
            tc.tile_pool(name="u", bufs=1) as up,
            tc.tile_pool(name="chi", bufs=1) as chip,
            tc.tile_pool(name="mj", bufs=2) as mjp,
            tc.tile_pool(name="sm", bufs=1) as smp,
            tc.tile_pool(name="ps", bufs=2, space="PSUM") as psp,
        ):
            tJ = constp.tile([128, 128], f32r, tag="jmat")
            nc.scalar.dma_start(tJ[:], jmat[:])
            # paired DFT tables: TTP[qp][:, sub, 512*i:512*(i+1)] holds table i
            # rows for m-chunk (qp + 2*sub)
            TTP = {}
            for qp in range(2):
                t = constp.tile([128, 2, 6 * 512], fp8, tag=f"ttp{qp}")
                TTP[qp] = t
                nc.gpsimd.dma_start(t[:], tabsp[qp])

            def tab(nm, qp):
                i = TABNAMES.index(nm)
                ap = TTP[qp][:]
                return bass.AP(ap.tensor, ap.offset + 512 * i,
                               [ap.ap[0], [6 * 512, 2], [1, 512]])

            warm = psp.tile([128, 1024], f32, tag="xe")

            def emit_load(b):
                s = {"b": b, "chis": [], "R": {}}
                scol = smp.tile([128, 24], f32, tag=f"scol{b}")
                nc.sync.dma_start(scol[:], scols[b])
                s["scol"] = scol
                ws = tp.tile([128, 2, W], bf16, name=f"ws{b}", tag=f"ws{b}")
                eng = nc.sync if b == 0 else nc.scalar
                eng.dma_start(
                    ws[:],
                    bass.AP(ds2, (b * 2) * DS_LEN + 384, [[1, 128], [DS_LEN, 2], [1, W]]),
                )
                s["ws"] = ws
                return s

            def emit_warm(s, n):
                # dummy matmuls to hold the PE HAM unthrottled before the real
                # matmul stream starts; reads the window tile, writes a scratch
                # PSUM bank.
                ws = s["ws"]
                ap = ws[:]
                l = bass.AP(ap.tensor, ap.offset, [ap.ap[0], [1, 128]])
                r = bass.AP(ap.tensor, ap.offset, [ap.ap[0], [1, 512]])
                for _ in range(n):
                    nc.tensor.matmul(warm[:, 0:512], l, r, start=True, stop=True)

            def emit_keepalive(dep_tile, mov_tile=None):
                # dummy matmul whose stationary reads a freshly-written tile,
                # so it lands spaced through the producing stream and keeps
                # the PE HAM active
                ap = dep_tile[:]
                l = bass.AP(ap.tensor, ap.offset, [ap.ap[0], [1, 128]])
                mp = (mov_tile if mov_tile is not None else TTP[0])[:]
                r = bass.AP(mp.tensor, mp.offset, [mp.ap[0], [1, 512]])
                nc.tensor.matmul(warm[:, 0:512], l, r, start=True, stop=True)

            def emit_alpha_act(b, s):
                # alpha = 1/sum_m |s[m]|^2: every partition of the sliding
                # window holds a full period, so a free-axis square-accumulate
                # over any 1024 columns yields the global sum per partition.
                ws = s["ws"]
                ap = ws[:]
                scr = tmpp.tile([128, 1024], f32, tag=f"asc{b % 2}")
                accR = smp.tile([128, 1], f32, tag=f"accR{b}")
                accI = smp.tile([128, 1], f32, tag=f"accI{b}")
                for blk, acc in ((0, accR), (1, accI)):
                    src = bass.AP(ap.tensor, ap.offset + blk * W + 512, [ap.ap[0], [1, 1024]])
                    nc.scalar.activation(scr[:], src, ACTF.Square, accum_out=acc[:])
                s["accs"] = (accR, accI)

            def emit_alpha_dve(b, s):
                accR, accI = s["accs"]
                ssum = smp.tile([128, 1], f32, tag=f"ssum{b}")
                nc.vector.tensor_add(ssum[:], accR[:], accI[:])
                alpha = smp.tile([128, 1], f32, tag=f"alpha{b}")
                nc.vector.reciprocal(alpha[:], ssum[:])
                s["alpha"] = alpha

            def emit_rbuild(s, lo, hi, keepalive=False):
                # R^T[m, kk] = s[m]*conj(s)[(m-kk)%N]; sum/diff of halves m and
                # m+512, written as fp8 into DoubleRow chunk-pair tiles.
                ws = s["ws"]
                wap = ws[:]
                scol = s["scol"]
                b = s["b"]
                n = hi - lo
                for q in range(4):
                    us = []
                    for h in (0, 1):
                        c8 = q + 4 * h
                        sr_c = scol[:, c8:c8 + 1]
                        si_c = scol[:, 8 + c8:9 + c8]
                        nsr_c = scol[:, 16 + c8:17 + c8]
                        j0 = 640 + 128 * q + 512 * h - lo
                        w_sr = bass.AP(wap.tensor, wap.offset + j0, [wap.ap[0], [-1, n]])
                        w_si = bass.AP(wap.tensor, wap.offset + W + j0, [wap.ap[0], [-1, n]])
                        a = tmpp.tile([128, 640], bf16, tag="ta")
                        ur = up.tile([128, 640], bf16, tag=f"ur{h}")
                        # Rr = sr_m*sr_win + si_m*si_win
                        nc.vector.tensor_scalar_mul(a[:, 0:n], w_sr, sr_c)
                        nc.vector.scalar_tensor_tensor(
                            ur[:, 0:n], w_si, si_c, a[:, 0:n], op0=ALU.mult, op1=ALU.add)
                        b2 = tmpp.tile([128, 640], bf16, tag="tb")
                        ui = up.tile([128, 640], bf16, tag=f"ui{h}")
                        # Ri = si_m*sr_win - sr_m*si_win
                        nc.vector.tensor_scalar_mul(b2[:, 0:n], w_si, nsr_c)
                        nc.vector.scalar_tensor_tensor(
                            ui[:, 0:n], w_sr, si_c, b2[:, 0:n], op0=ALU.mult, op1=ALU.add)
                        if keepalive:
                            emit_keepalive(ui, mov_tile=ws)
                        us.append((ur, ui))
                    (u1r, u1i), (u2r, u2i) = us
                    qp, sub = q % 2, q // 2
                    if lo == 0:
                        for nm in ("rsr", "rsi", "rdr", "rdi"):
                            if (nm, qp) not in s["R"]:
                                s["R"][(nm, qp)] = rp.tile(
                                    [128, 2, 640], fp8,
                                    name=f"{nm}{qp}b{b}", tag=f"{nm}{qp}_{b % 2}",
                                )
                    def rsl(nm):
                        ap = s["R"][(nm, qp)][:]
                        return bass.AP(ap.tensor, ap.offset + sub * 640 + lo, [ap.ap[0], [1, n]])
                    nc.vector.tensor_add(rsl("rsr"), u1r[:, 0:n], u2r[:, 0:n])
                    nc.vector.tensor_sub(rsl("rdr"), u1r[:, 0:n], u2r[:, 0:n])
                    nc.vector.tensor_add(rsl("rsi"), u1i[:, 0:n], u2i[:, 0:n])
                    nc.vector.tensor_sub(rsl("rdi"), u1i[:, 0:n], u2i[:, 0:n])
                    if keepalive:
                        emit_keepalive(s["R"][("rdi", qp)])

            def rweights(s, nm, qp, c):
                ap = s["R"][(nm, qp)][:]
                return bass.AP(ap.tensor, ap.offset + c, [ap.ap[0], [640, 2], [1, 128]])

            def emit_kblock(b, s, kb):
                c = 128 * kb
                xe = psp.tile([128, 1024], f32, tag="xe")
                xo = psp.tile([128, 1024], f32, tag="xo")
                xre, xie = xe[:, 0:512], xe[:, 512:1024]
                xro, xio = xo[:, 0:512], xo[:, 512:1024]
                for qp in range(2):
                    first = qp == 0
                    last = qp == 1
                    psr = rweights(s, "rsr", qp, c)
                    psi = rweights(s, "rsi", qp, c)
                    pdr = rweights(s, "rdr", qp, c)
                    pdi = rweights(s, "rdi", qp, c)
                    mm = lambda o, l, r, st, sp: nc.tensor.matmul(
                        o, l, r, start=st, stop=sp, perf_mode=PM.DoubleRow)
                    # xe's groups close first so its square starts while xo's
                    # last matmuls still stream
                    mm(xre, psr, tab("tec", qp), first, False)
                    mm(xie, psi, tab("tec", qp), first, False)
                    mm(xre, psi, tab("tes", qp), False, last)
                    mm(xie, psr, tab("tesn", qp), False, last)
                    mm(xro, pdr, tab("toc", qp), first, False)
                    mm(xio, pdi, tab("toc", qp), first, False)
                    mm(xro, pdi, tab("tos", qp), False, last)
                    mm(xio, pdr, tab("tosn", qp), False, last)

                chi_t = chip.tile([128, N], f32r, tag=f"chi{(5 * b + kb) % 6}")
                alpha = s["alpha"]
                for parity, x2 in ((0, xe), (1, xo)):
                    sq = tmpp.tile([128, 1024], f32, tag=f"sq{parity}")
                    # chi = (alpha*xr)^2 + (alpha*xi)^2: normalization folded
                    # into the activation scale
                    nc.scalar.activation(sq[:], x2[:], ACTF.Square, scale=alpha[:])
                    cap = chi_t[:]
                    strided = bass.AP(cap.tensor, cap.offset + parity, [cap.ap[0], [2, 512]])
                    nc.gpsimd.tensor_add(strided, sq[:, 0:512], sq[:, 512:1024])
                s["chis"].append(chi_t)

            def emit_direct(b, s, kbs):
                for kb in kbs:
                    r0 = (128 * kb + 512) % N
                    nc.sync.dma_start(out[b, r0:r0 + 128, :], s["chis"][kb][:].bitcast(f32))

            def emit_mirror_jcopy(b, s, kbs):
                # k-flip: J matmul on chi reverses partitions; the f-reversal
                # rides the PSUM->SBUF copies. ACT handles batch 0 (DVE busy
                # with rbuild(s1)); DVE handles batch 1 (idle by then).
                s.setdefault("mj", {})
                ceng = nc.scalar if b == 0 else nc.vector
                cp = ceng.copy if b == 0 else ceng.tensor_copy
                for kb in kbs:
                    chi_t = s["chis"][kb]
                    jy = psp.tile([128, 1024], f32, tag="xe")
                    nc.tensor.matmul(jy[:, 0:512], tJ[:],
                                     chi_t[:, 0:512], start=True, stop=True)
                    nc.tensor.matmul(jy[:, 512:1024], tJ[:],
                                     chi_t[:, 512:1024], start=True, stop=True)
                    mj = mjp.tile([128, N], f32, tag=f"mj{kb % 2}")
                    jap = jy[:]
                    rev_hi = bass.AP(jap.tensor, jap.offset + 1023, [jap.ap[0], [-1, 511]])
                    rev_lo = bass.AP(jap.tensor, jap.offset + 511, [jap.ap[0], [-1, 511]])
                    cp(mj[:, 0:1], jy[:, 0:1])
                    cp(mj[:, 1:512], rev_hi)
                    cp(mj[:, 512:513], jy[:, 512:513])
                    cp(mj[:, 513:1024], rev_lo)
                    s["mj"][kb] = mj

            def emit_mirror_store(b, s, kbs):
                # mj partition r holds k = c+127-r -> dest row 385-c+r
                for kb in kbs:
                    c = 128 * kb
                    mj = s["mj"][kb]
                    eng = nc.scalar
                    if kb == 0:
                        eng.dma_start(out[b, 385:512, :], mj[0:127, :])
                    elif kb == 3:
                        eng.dma_start(out[b, 128:129, :], mj[127:128, :])
                    else:
                        r0 = 385 - c
                        eng.dma_start(out[b, r0:r0 + 128, :], mj[:])

            # --- pipelined schedule: DVE runs the two rbuilds back-to-back;
            # the PE streams batch-0 kblocks against rbuild(s1); warm/keepalive
            # matmuls bridge the PE-idle stretches so the HAM stays at 8/8.
            s0 = emit_load(0)
            s1 = emit_load(1)
            emit_warm(s0, 14)
            emit_alpha_act(0, s0)
            emit_alpha_act(1, s1)
            emit_rbuild(s0, 0, 640, keepalive=True)
            emit_alpha_dve(0, s0)
            emit_kblock(0, s0, 0)
            emit_direct(0, s0, [0])
            emit_alpha_dve(1, s1)
            emit_rbuild(s1, 0, 640)
            emit_kblock(0, s0, 1)
            emit_direct(0, s0, [1])
            emit_mirror_jcopy(0, s0, [0])
            emit_mirror_store(0, s0, [0])
            emit_kblock(0, s0, 2)
            emit_direct(0, s0, [2])
            emit_mirror_jcopy(0, s0, [1])
            emit_mirror_store(0, s0, [1])
            emit_kblock(0, s0, 3)
            emit_direct(0, s0, [3])
            emit_mirror_jcopy(0, s0, [2])
            emit_mirror_store(0, s0, [2])
            emit_kblock(0, s0, 4)
            emit_direct(0, s0, [4])
            emit_mirror_jcopy(0, s0, [3])
            emit_mirror_store(0, s0, [3])
            # bridge the gap until rbuild(s1) completes, one keepalive per
            # R array so the deps land spread across its combine stream
            for nm in ("rsr", "rsi", "rdr", "rdi"):
                ap = s1["R"][(nm, 1)][:]
                l = bass.AP(ap.tensor, ap.offset, [ap.ap[0], [1, 128]])
                tp_ap = TTP[0][:]
                r = bass.AP(tp_ap.tensor, tp_ap.offset, [tp_ap.ap[0], [1, 512]])
                nc.tensor.matmul(warm[:, 0:512], l, r, start=True, stop=True)
            emit_kblock(1, s1, 0)
            emit_direct(1, s1, [0])
            for kb in range(1, 5):
                emit_kblock(1, s1, kb)
                emit_direct(1, s1, [kb])
                emit_mirror_jcopy(1, s1, [kb - 1])
                emit_mirror_store(1, s1, [kb - 1])

    _split_excess_waits(nc)
    return nc


_NC_CACHE = {}


def _get_nc():
    if "nc" not in _NC_CACHE:
        _NC_CACHE["nc"] = build_nc()
    return _NC_CACHE["nc"]


def _get_tables():
    if "tabs" not in _NC_CACHE:
        m = np.arange(512, dtype=np.float64)[:, None]
        tp_ = np.arange(512, dtype=np.float64)[None, :]
        t_of = (tp_ + 256) % 512
        ang_e = 2.0 * np.pi * ((m * t_of) % 512) / 512
        ang_o = ang_e + 2.0 * np.pi * m / 1024
        tabs = {
            "tec": np.cos(ang_e),
            "tes": np.sin(ang_e),
            "toc": np.cos(ang_o),
            "tos": np.sin(ang_o),
        }
        tabs["tesn"] = -tabs["tes"]
        tabs["tosn"] = -tabs["tos"]
        # paired fp8 layout: tabsp[qp, p, sub, 512*i+t] = tab_i[128*(qp+2*sub)+p, t]
        tabsp = np.zeros((2, 128, 2, 6 * 512), dtype=np.float64)
        for i, nm in enumerate(TABNAMES):
            tq = tabs[nm].reshape(4, 128, 512)  # [chunk, p, t]
            for qp in range(2):
                for sub in range(2):
                    tabsp[qp, :, sub, 512 * i:512 * (i + 1)] = tq[qp + 2 * sub]
        _NC_CACHE["tabs"] = (
            tabsp.astype(ml_dtypes.float8_e4m3),
            np.eye(128, dtype=np.float32)[::-1].copy(),
        )
    return _NC_CACHE["tabs"]


def make_in_maps(s_real: np.ndarray, s_imag: np.ndarray):
    s_real = np.asarray(s_real, dtype=np.float32)
    s_imag = np.asarray(s_imag, dtype=np.float32)
    tabsp, jnp_ = _get_tables()
    in_maps = []
    for core in range(NCORES):
        sl = slice(core * BPC, (core + 1) * BPC)
        sr = s_real[sl].astype(ml_dtypes.bfloat16)
        si = s_imag[sl].astype(ml_dtypes.bfloat16)
        ds2 = np.stack(
            [np.tile(sr, (1, 3))[:, :DS_LEN], np.tile(si, (1, 3))[:, :DS_LEN]],
            axis=1,
        ).copy()
        scols = np.concatenate(
            [
                sr.reshape(BPC, 8, 128).transpose(0, 2, 1),
                si.reshape(BPC, 8, 128).transpose(0, 2, 1),
                (-sr).reshape(BPC, 8, 128).transpose(0, 2, 1),
            ],
            axis=2,
        ).astype(np.float32).copy()
        im = {"ds2": ds2, "scols": scols, "tabsp": tabsp, "jmat": jnp_}
        in_maps.append(im)
    return in_maps


def kernel(s_real: np.ndarray, s_imag: np.ndarray) -> np.ndarray:
    nc = _get_nc()
    in_maps = make_in_maps(s_real, s_imag)
    res = bass_utils.run_bass_kernel_spmd(nc, in_maps, core_ids=list(range(NCORES)))
    return np.concatenate([r["out"] for r in res.results], axis=0)
